# revision 13
# baseline (speedup 1.0000x reference)
"""EEGTokenizer Trainium2 kernel: 8-core data-parallel Bass/Tile implementation.

Full inputs in, full output out. Internally shards batch (32) as 4 per core.
"""
import sys
sys.path.insert(0, "/opt/trn_rl_repo")

import numpy as np
from contextlib import ExitStack

import concourse.bass as bass
import concourse.tile as tile
from concourse import bacc, mybir
from concourse.bass_utils import run_bass_kernel_spmd
from concourse.masks import make_identity

FP32 = mybir.dt.float32
FP32R = mybir.dt.float32r
BF16 = mybir.dt.bfloat16
AF = mybir.ActivationFunctionType
ALU = mybir.AluOpType

# problem dims (hardcoded per spec)
B, T, C, D = 32, 4096, 256, 256
L = 128          # output seq len
W = 256          # fft window
NWIN = 31        # sliding windows
NBINS = 50       # freq bins k=1..50 cover all bands
NB = 4           # batch elems per core
NCORES = 8
EPS = 1e-5
BANDS = [(0.5, 4.0), (4.0, 8.0), (8.0, 13.0), (13.0, 30.0), (30.0, 50.0)]
# 0-based bin slices (k-1) within k=1..50
BAND_SLICES = [(0, 4), (3, 8), (7, 13), (12, 30), (29, 50)]
BAND_CNT = [4, 5, 6, 18, 21]

TT = 512         # conv T-tile
NTT = T // TT    # 8
XPAD_L = 15      # left pad in X2 (max K//2)
X2W = XPAD_L + T + 17   # 4128

# conv branch order matches concat: [t3, t7, t15, t31]
BRANCH_K = [3, 7, 15, 31]
BRANCH_BASE = [0, 64, 128, 192]


def _conv_slot_table():
    """Pair-packed conv matmul slots.

    Each slot: (x2_offset, bank, psum_base, width, [(branch, col0, tap0), ...])
    lhsT rows: [ci + 64*h] for h in {0,1} -> taps (tap0+h) relative to branch.
    """
    slots = []
    # all slots full 128-wide at psum base 0 (col-offset tiling rejected by
    # walrus for f32r); solo-branch slots carry zero weights in the other half
    # bank0: t3 (psum 0:64) + t7 (64:128)
    slots.append((14, 0, 0, 128, [(0, 0, 0), (1, 64, 2)]))   # t3 taps0,1 ; t7 taps2,3
    slots.append((16, 0, 0, 128, [(0, 0, 2), (1, 64, 4)]))   # t3 tap2(+pad) ; t7 taps4,5
    slots.append((12, 0, 0, 128, [(1, 64, 0)]))              # t7 taps0,1 (t3 half zero)
    slots.append((18, 0, 0, 128, [(1, 64, 6)]))              # t7 tap6 (t3 half zero)
    # bank1: t15 (psum 0:64) + t31 (64:128)
    for i, o in enumerate(range(8, 24, 2)):
        slots.append((o, 1, 0, 128, [(2, 0, 2 * i), (3, 64, o)]))
    for o in (0, 2, 4, 6, 24, 26, 28, 30):
        slots.append((o, 1, 0, 128, [(3, 64, o)]))           # t31 only (t15 half zero)
    return slots


CONV_SLOTS = _conv_slot_table()


def _prep_consts(inp):
    """Host-side constant folding. Returns dict name -> np array."""
    f32 = np.float32
    out = {}
    s1 = (inp["g1"] / np.sqrt(inp["v1"] + EPS)).astype(np.float64)
    s2 = (inp["g2"] / np.sqrt(inp["v2"] + EPS)).astype(np.float64)
    assert (s2 > 0).all(), "BN2 scale must be positive for relu fold"
    c2 = (inp["be2"] - inp["m2"] * s2).astype(np.float64)

    # stem: (256,64) lhsT with BN1 fold; tile (128,128) col-blocks per C-chunk
    Wst = (inp["w_sp"][:, :, 0].astype(np.float64) * s1[:, None]).T  # (256,64)
    bst = ((inp["b_sp"] - inp["m1"]) * s1 + inp["be1"])
    wstem = np.zeros((128, 128), f32)
    wstem[:, 0:64] = Wst[0:128]
    wstem[:, 64:128] = Wst[128:256]
    out["wstem"] = wstem
    out["bstem"] = np.asarray(bst, f32).reshape(64, 1)

    # conv slots -> (128, total_width), weights scaled by s2[out_ch]
    wts = [inp["wt3"], inp["wt7"], inp["wt15"], inp["wt31"]]
    bts = [inp["bt3"], inp["bt7"], inp["bt15"], inp["bt31"]]
    tot = sum(s[3] for s in CONV_SLOTS)
    wconv = np.zeros((128, tot), np.float64)
    col = 0
    for (o, bank, p0, width, brs) in CONV_SLOTS:
        for (br, c0, tap0) in brs:
            K = BRANCH_K[br]
            wb = wts[br].astype(np.float64)  # (64co, 64ci, K)
            for h in (0, 1):
                k = tap0 + h
                if k < K:
                    gch = BRANCH_BASE[br]
                    sc = s2[gch:gch + 64]
                    wconv[64 * h:64 * h + 64, col + c0: col + c0 + 64] = \
                        wb[:, :, k].T * sc[None, :]
        col += width
    out["wconv"] = wconv.astype(f32)
    bconv = np.concatenate([np.asarray(b, np.float64) for b in bts]) * s2
    # bank-partition layout: bank0 = ch 0:128 (t3,t7), bank1 = ch 128:256
    out["bconv0"] = bconv[0:128].astype(f32).reshape(128, 1)
    out["bconv1"] = bconv[128:256].astype(f32).reshape(128, 1)

    # DFT matrices, bf16: (128, 200): cols 0:100 = rows 0:128, 100:200 = rows 128:256
    tau = np.arange(W, dtype=np.float64)[:, None]
    k = np.arange(1, NBINS + 1, dtype=np.float64)[None, :]
    ang = 2.0 * np.pi * k * tau / W
    FRI = np.concatenate([np.cos(ang), -np.sin(ang)], axis=1)  # (256, 100)
    fdft = np.zeros((128, 200), np.float64)
    fdft[:, 0:100] = FRI[0:128]
    fdft[:, 100:200] = FRI[128:256]
    out["fdft"] = fdft.astype(np.dtype(mybir.dt.np(BF16)))

    # pool ones (128, 4): block mean 1/32
    po = np.zeros((128, 4), np.float64)
    for j in range(4):
        po[32 * j:32 * j + 32, j] = 1.0 / 32.0
    out["pones"] = po.astype(np.dtype(mybir.dt.np(BF16)))

    # wp1 with band-mean fold: (1280,128)/cnt -> tile (128, 1280) col-blocks
    wp1 = inp["wp1"].astype(np.float64).copy()  # (1280, 128)
    for b_i in range(5):
        wp1[b_i * 256:(b_i + 1) * 256] /= BAND_CNT[b_i]
    w1t = np.zeros((128, 1280), np.float64)
    for j in range(10):
        w1t[:, 128 * j:128 * (j + 1)] = wp1[128 * j:128 * (j + 1)]
    out["wp1e"] = w1t.astype(f32)
    out["bp1"] = np.asarray(inp["bp1"], f32).reshape(128, 1)
    out["wp2"] = np.asarray(inp["wp2"], f32)  # (128, 256)

    # pooled-interp matrix P~^T replicated at partition bases {0,32,64,96}
    src = np.clip((np.arange(T) + 0.5) * (NWIN / T) - 0.5, 0.0, NWIN - 1.0)
    i0 = np.floor(src).astype(np.int64)
    i1 = np.minimum(i0 + 1, NWIN - 1)
    wg = src - i0
    IM = np.zeros((T, NWIN), np.float64)
    IM[np.arange(T), i0] += 1.0 - wg
    IM[np.arange(T), i1] += wg
    PM = IM.reshape(L, 32, NWIN).mean(axis=1)  # (128, 31)
    pp = np.zeros((128, 128), np.float64)
    for b_i in range(4):
        pp[32 * b_i:32 * b_i + 31, :] = PM.T
    out["ppool"] = pp.astype(f32)

    # fusion weights + folded bias
    wfu = inp["w_fu"].astype(np.float64)  # (512, 256)
    bfu = (inp["b_fu"].astype(np.float64)
           + wfu[0:256].T @ c2
           + wfu[256:512].T @ inp["bp2"].astype(np.float64))  # (256,)
    wfut = np.zeros((128, 1024), np.float64)
    for kk in range(4):
        for d in range(2):
            wfut[:, (2 * kk + d) * 128:(2 * kk + d) * 128 + 128] = \
                wfu[128 * kk:128 * kk + 128, 128 * d:128 * d + 128]
    out["wfu"] = wfut.astype(f32)
    # posT + b_fu_eff: (256, 128) -> tile (128, 256) col-blocks per D-chunk
    posT = inp["pos"][0, :L, :].astype(np.float64).T + bfu[:, None]  # (256,128)
    ptb = np.zeros((128, 256), np.float64)
    ptb[:, 0:128] = posT[0:128]
    ptb[:, 128:256] = posT[128:256]
    out["posb"] = ptb.astype(f32)

    # output projection
    wout = np.zeros((128, 512), np.float64)
    wo = inp["w_out"].astype(np.float64)  # (256,256)
    wout[:, 0:256] = wo[0:128]
    wout[:, 256:512] = wo[128:256]
    out["wout"] = wout.astype(f32)
    out["bout"] = np.asarray(inp["b_out"], f32).reshape(1, 256)
    out["onerow"] = np.full((1, 128), 1.0, f32)
    return out


CONST_SPECS = [
    ("wstem", (128, 128), FP32R), ("bstem", (64, 1), FP32),
    ("wconv", (128, sum(s_[3] for s_ in CONV_SLOTS)), FP32R), ("bconv0", (128, 1), FP32), ("bconv1", (128, 1), FP32),
    ("fdft", (128, 200), BF16), ("pones", (128, 4), BF16),
    ("wp1e", (128, 1280), FP32), ("bp1", (128, 1), FP32), ("wp2", (128, 256), FP32),
    ("ppool", (128, 128), FP32), ("wfu", (128, 1024), FP32), ("posb", (128, 256), FP32),
    ("wout", (128, 512), FP32R), ("bout", (1, 256), FP32R), ("onerow", (1, 128), FP32R),
]


def build_program(debug=False):
    nc = bacc.Bacc("TRN2", target_bir_lowering=False, debug=False)
    xs = nc.dram_tensor("xs", [NB, T, C], FP32, kind="ExternalInput")
    cin = {n: nc.dram_tensor(n, list(sh), dt, kind="ExternalInput")
           for (n, sh, dt) in CONST_SPECS}
    ys = nc.dram_tensor("ys", [NB, L, D], FP32, kind="ExternalOutput")
    dbg = {}
    if debug:
        for nm, sh, dt_ in [("dX2", [128, X2W], FP32), ("dxc2", [256, T], BF16),
                            ("dxc2T", [128, 32 * 256], BF16),
                            ("dpsri", [2, 128, NWIN * 100], BF16),
                            ("dpsq", [2, 128, NB * NWIN * NBINS], BF16),
                            ("dfeatsT", [128, 10 * NB * NWIN], FP32),
                            ("dh1rT", [128, 128], FP32), ("dhpad", [128, 256], FP32),
                            ("dfrhs", [128, 512], FP32), ("dpp", [128, 256], FP32)]:
            dbg[nm] = nc.dram_tensor(nm, sh, dt_, kind="ExternalOutput")

    with tile.TileContext(nc) as tc, ExitStack() as ctx:
        cpool = ctx.enter_context(tc.tile_pool(name="consts", bufs=1))
        xpool = ctx.enter_context(tc.tile_pool(name="xtiles", bufs=8))
        big = ctx.enter_context(tc.tile_pool(name="big", bufs=1))
        spool = ctx.enter_context(tc.tile_pool(name="small", bufs=2))
        sqpool = ctx.enter_context(tc.tile_pool(name="psqri", bufs=2))
        frpool = ctx.enter_context(tc.tile_pool(name="frhs", bufs=NB))
        pspool = ctx.enter_context(tc.tile_pool(name="ps", bufs=6, space="PSUM"))

        # --- consts into SBUF ---
        ct = {}
        for (n, sh, dt) in CONST_SPECS:
            t_ = cpool.tile([sh[0], sh[1]], dt, tag=f"c_{n}", name=f"c_{n}")
            nc.sync.dma_start(t_[:], cin[n][:, :])
            ct[n] = t_
        ident = cpool.tile([128, 128], FP32, tag="identf")
        make_identity(nc, ident[:])
        identb = cpool.tile([128, 128], BF16, tag="identb")
        nc.vector.tensor_copy(identb[:], ident[:])

        def mm32r(out, lhsT, rhs, **kw):
            nc.tensor.matmul(out, lhsT, rhs, **kw)

        # --- persistent per-core tiles ---
        psq = [big.tile([128, NB * NWIN * NBINS], BF16, tag=f"psq{c}", name=f"psq{c}")
               for c in range(2)]
        featsT = big.tile([128, 10 * NB * NWIN], FP32, tag="featsT")
        frhs = []   # per-batch fusion rhs (128 x 512): [poolc0|poolc1|spec0|spec1]

        for b in range(NB):
            # ---------- load + transpose x ----------
            xT = [big.tile([128, T], FP32R, tag=f"xT{c}", name=f"xT{c}") for c in range(2)]
            for s in range(8):
                xt4 = [xpool.tile([128, C], FP32, tag="xt", name="xt") for _ in range(4)]
                for i in range(4):
                    nc.sync.dma_start(xt4[i][:], xs[b, 128 * (4 * s + i):128 * (4 * s + i) + 128, :])
                for c in range(2):
                    ptr = pspool.tile([128, 512], FP32, tag="ps", name="ptr")
                    for i in range(4):
                        nc.tensor.transpose(ptr[:, 128 * i:128 * i + 128],
                                            xt4[i][:, 128 * c:128 * c + 128], ident[:])
                    nc.scalar.copy(xT[c][:, 512 * s:512 * s + 512], ptr[:])

            # ---------- stem + BN1 + relu -> X2 ----------
            X2 = big.tile([128, X2W], FP32R, tag="X2")
            nc.vector.memset(X2[0:64, 0:XPAD_L].bitcast(FP32), 0.0)
            nc.vector.memset(X2[0:64, XPAD_L + T:X2W].bitcast(FP32), 0.0)
            for s in range(NTT):
                pst = pspool.tile([128, 512], FP32, tag="ps", name="pst")
                for c in range(2):
                    mm32r(pst[0:64, :], ct["wstem"][:, 64 * c:64 * c + 64],
                          xT[c][:, TT * s:TT * s + TT],
                          start=(c == 0), stop=(c == 1))
                nc.vector.tensor_scalar(
                    out=X2[0:64, XPAD_L + TT * s:XPAD_L + TT * s + TT],
                    in0=pst[0:64, :], scalar1=ct["bstem"][:, 0:1], scalar2=0.0,
                    op0=ALU.add, op1=ALU.max)
            nc.vector.tensor_copy(X2[64:128, 0:X2W - 1], X2[0:64, 1:X2W])

            # ---------- temporal convs + BN2(+relu) -> xc2 (bf16) ----------
            xc2 = [big.tile([128, T], BF16, tag=f"xc2{c}", name=f"xc2{c}") for c in range(2)]
            for s in range(NTT):
                pcv = [pspool.tile([128, 512], FP32, tag="ps", name="pcv") for _ in range(2)]
                started = [False, False]
                col = 0
                for (o, bank, p0, width, brs) in CONV_SLOTS:
                    mm32r(pcv[bank][p0:p0 + width, :],
                          ct["wconv"][:, col:col + width],
                          X2[:, o + TT * s:o + TT * s + TT],
                          start=not started[bank], stop=False)
                    started[bank] = True
                    col += width
                for bank in range(2):
                    nc.scalar.activation(
                        xc2[bank][:, TT * s:TT * s + TT], pcv[bank][:],
                        AF.Relu, bias=ct["bconv0" if bank == 0 else "bconv1"][:, 0:1])

            # ---------- transpose xc2 -> xc2T (bf16) ----------
            xc2T = big.tile([128, 32 * 256], BF16, tag="xc2T")
            x2Tv = xc2T[:].rearrange("p (t x) -> p t x", x=256)
            for c in range(2):
                for s in range(8):
                    ptr = pspool.tile([128, 512], BF16, tag="psb", name="ptrb", bufs=2)
                    for i in range(4):
                        nc.tensor.transpose(ptr[:, 128 * i:128 * i + 128],
                                            xc2[c][:, 128 * (4 * s + i):128 * (4 * s + i) + 128],
                                            identb[:])
                    nc.vector.tensor_copy(x2Tv[:, 4 * s:4 * s + 4, 128 * c:128 * c + 128],
                                          ptr[:].rearrange("p (i x) -> p i x", x=128))

            # ---------- DFT windows + pooling (PE) ----------
            psri = [sqpool.tile([128, NWIN * 100], BF16, tag=f"psri{c}", name=f"psri{c}") for c in range(2)]
            pspool_b = [pspool.tile([128, 512], FP32, tag="ps", name="pspb") for _ in range(2)]
            ngroups = (NWIN + 4) // 5  # 7
            pgrp = [[None] * ngroups for _ in range(2)]
            for t in range(32):
                for c in range(2):
                    lhs = xc2T[:, 256 * t + 128 * c:256 * t + 128 * c + 128]
                    # F1 for window t-1 MUST precede F0 for window t: the
                    # start=True below clears the whole bank's has_written bits
                    if t >= 1:
                        g = (t - 1) // 5
                        nc.tensor.matmul(pgrp[c][g][:, 100 * ((t - 1) % 5):100 * ((t - 1) % 5) + 100],
                                         lhs, ct["fdft"][:, 100:200],
                                         start=False, stop=True)
                    if t <= 30:
                        g = t // 5
                        if pgrp[c][g] is None:
                            pgrp[c][g] = pspool.tile([128, 500], FP32, tag="ps", name="pdft")
                        nc.tensor.matmul(pgrp[c][g][:, 100 * (t % 5):100 * (t % 5) + 100],
                                         lhs, ct["fdft"][:, 0:100],
                                         start=True, stop=False)
                    if t >= 1:
                        g = (t - 1) // 5
                        if (t - 1) % 5 == 4 or (t - 1) == 30:
                            g0 = (t - 1) // 5
                            n_w = min(5, NWIN - 5 * g0)
                            nc.scalar.activation(
                                psri[c][:, 500 * g0:500 * g0 + 100 * n_w],
                                pgrp[c][g0][:, 0:100 * n_w], AF.Square)
                    # adaptive pool of xc2 via same stationary
                    nc.tensor.matmul(pspool_b[c][:, 4 * t:4 * t + 4], lhs,
                                     ct["pones"][:], start=(t == 0), stop=(t == 31))

            if debug and b == 0:
                nc.sync.dma_start(dbg["dX2"][:, :], X2[:].bitcast(FP32))
                for c in range(2):
                    nc.sync.dma_start(dbg["dxc2"][128 * c:128 * c + 128, :], xc2[c][:])
                nc.sync.dma_start(dbg["dxc2T"][:, :], xc2T[:])
                for c in range(2):
                    nc.sync.dma_start(dbg["dpsri"][c, :, :], psri[c][:])
            fr = frpool.tile([128, 512], FP32, tag="frhs")
            frhs.append(fr)
            for c in range(2):
                nc.vector.tensor_copy(fr[:, 128 * c:128 * c + 128], pspool_b[c][:, 0:128])

            # ---------- re^2+im^2 ----------
            for c in range(2):
                a3 = psri[c][:].rearrange("p (w r) -> p w r", r=100)
                o3 = psq[c][:, NWIN * NBINS * b:NWIN * NBINS * (b + 1)] \
                    .rearrange("p (w k) -> p w k", k=NBINS)
                nc.vector.tensor_tensor(out=o3, in0=a3[:, :, 0:50],
                                        in1=a3[:, :, 50:100], op=ALU.add)

        if debug:
            for c in range(2):
                nc.sync.dma_start(dbg["dpsq"][c, :, :], psq[c][:])
        # ---------- band reduce -> featsT ----------
        for bd in range(5):
            lo, hi = BAND_SLICES[bd]
            for c in range(2):
                src = psq[c][:].rearrange("p (w k) -> p w k", k=NBINS)[:, :, lo:hi]
                j = bd * 2 + c
                nc.vector.tensor_reduce(
                    out=featsT[:, 124 * j:124 * j + 124], in_=src,
                    axis=mybir.AxisListType.X, op=ALU.add)

        if debug:
            nc.sync.dma_start(dbg["dfeatsT"][:, :], featsT[:])
        # ---------- MLP ----------
        ph1 = pspool.tile([128, 124], FP32, tag="ps")
        for j in range(10):
            nc.tensor.matmul(ph1[:], ct["wp1e"][:, 128 * j:128 * j + 128],
                             featsT[:, 124 * j:124 * j + 124],
                             start=(j == 0), stop=(j == 9))
        h1rT = spool.tile([128, 128], FP32, tag="h1rT")
        nc.gpsimd.memset(h1rT[:], 0.0)
        nc.scalar.activation(
            h1rT[:].rearrange("p (b w) -> p b w", w=32)[:, :, 0:31],
            ph1[:].rearrange("p (b w) -> p b w", w=31),
            AF.Relu, bias=ct["bp1"][:, 0:1])
        php = pspool.tile([128, 256], FP32, tag="ps")
        nc.tensor.matmul(php[:], h1rT[:], ct["wp2"][:], start=True, stop=True)
        hpad = spool.tile([128, 256], FP32, tag="hpad")
        nc.vector.tensor_copy(hpad[:], php[:])

        if debug:
            nc.sync.dma_start(dbg["dh1rT"][:, :], h1rT[:])
            nc.sync.dma_start(dbg["dhpad"][:, :], hpad[:])
        # ---------- per-batch tail: interp, fusion, pos, out-proj ----------
        for b in range(NB):
            for d in range(2):
                psp = pspool.tile([128, 128], FP32, tag="ps")
                nc.tensor.matmul(psp[:], hpad[32 * b:32 * b + 31, 128 * d:128 * d + 128],
                                 ct["ppool"][32 * b:32 * b + 31, :],
                                 start=True, stop=True, tile_position=(32 * b, 0))
                nc.vector.tensor_copy(frhs[b][:, 256 + 128 * d:256 + 128 * d + 128], psp[:])
            pp = spool.tile([128, 256], FP32R, tag="pp")
            for d in range(2):
                pfu = pspool.tile([128, 128], FP32, tag="ps")
                for kk in range(4):
                    nc.tensor.matmul(pfu[:], ct["wfu"][:, (2 * kk + d) * 128:(2 * kk + d) * 128 + 128],
                                     frhs[b][:, 128 * kk:128 * kk + 128],
                                     start=(kk == 0), stop=(kk == 3))
                nc.vector.tensor_tensor(out=pp[:, 128 * d:128 * d + 128], in0=pfu[:],
                                        in1=ct["posb"][:, 128 * d:128 * d + 128],
                                        op=ALU.add)
            py = pspool.tile([128, 256], FP32, tag="ps")
            for d in range(2):
                mm32r(py[:], pp[:, 128 * d:128 * d + 128],
                      ct["wout"][:, 256 * d:256 * d + 256],
                      start=(d == 0), stop=False)
            mm32r(py[:], ct["onerow"][0:1, :], ct["bout"][0:1, :],
                  start=False, stop=True)
            if debug and b == 0:
                nc.sync.dma_start(dbg["dfrhs"][:, :], frhs[b][:])
                nc.sync.dma_start(dbg["dpp"][:, :], pp[:].bitcast(FP32))
            ysb = spool.tile([128, 256], FP32, tag="ysb")
            nc.vector.tensor_copy(ysb[:], py[:])
            nc.sync.dma_start(ys[b, :, :], ysb[:])

    nc.compile()
    return nc


_NC_CACHE = None


def kernel(**inputs):
    global _NC_CACHE
    if _NC_CACHE is None:
        _NC_CACHE = build_program()
    nc = _NC_CACHE
    consts = _prep_consts(inputs)
    x = np.ascontiguousarray(inputs["x"], dtype=np.float32)
    in_maps = []
    for i in range(NCORES):
        m = dict(consts)
        m["xs"] = x[NB * i:NB * (i + 1)]
        in_maps.append(m)
    res = run_bass_kernel_spmd(nc, in_maps, list(range(NCORES)))
    return np.concatenate([res.results[i]["ys"] for i in range(NCORES)], axis=0)
